# revision 14
# baseline (speedup 1.0000x reference)
"""Trainium2 Bass kernel: per-(head,batch) euclidean compatibility matrix,
globally min/max-rescaled to [-9, 9].

reference (jax):
    q_sq = sum(Q*Q, -1)[..., :, None]
    k_sq = sum(K*K, -1)[..., None, :]
    cross = einsum("hbqd,hbgd->hbqg", Q, K)
    compat = sqrt(q_sq + k_sq - 2*cross)
    out = A_LO + (compat - min) * (A_HI - A_LO) / (max - min)   # min/max per (h,b)

Sharding: head h -> NeuronCore h (8 heads, 8 cores), fully independent.

Per-core program (B=4 slices of [N=2048, D=16]):
  - load Q[b]/K[b] in natural layout, augment each 128-row chunk with
    (ones, row-sum-of-squares) columns, PE-transpose to build
      UT = [Q^T; 1; q_sq]  (18 x 2048)     VT = [-2*K^T; k_sq; 1]  (18 x 2048)
    so that d2 = UT[:,q]^T @ VT[:,g] is the squared euclidean distance.
  - per 128-row q-tile: f32r matmul -> PSUM d2, ACT sqrt -> SBUF sq,
    fused DVE tensor_scalar+accum reduces (min / max partials).
  - finalize min/max across tiles + partitions (gpsimd all-reduce),
    compute c1 = 18/(max-min), c0 = -9 - min*c1 broadcast per partition.
  - per q-tile: out = sq*c1 + c0 (DVE / ACT split), DMA to DRAM.
"""

import numpy as np

H, B, N, D = 8, 4, 2048, 16
A_LO, A_HI = -9.0, 9.0
P = 128
NT = N // P          # 16 q-tiles per slice
HALF = N // 2        # PSUM d2 tile width (2 banks)

# ---- tuning knobs ----
USE_F32R = True      # f32r matmul: 1 cyc/row vs 4 for plain f32
ACT_MADDS = 6        # of NT final-affine ops per slice routed to ACT (rest DVE)
SQ_BUFS = 17         # SBUF slots of [128, 2048] f32 for sq tiles
FUSED_REDUCE = True  # tensor_scalar+accum_out (2x mode) vs tensor_reduce (1x)
PHASE_MODE = "full"  # debug: "nored" (no reduces/C/madd), "nomadd" (no madd)

_CACHE = {}


def build_program():
    import concourse.bacc as bacc
    import concourse.bass as bass
    import concourse.mybir as mybir
    from concourse import tile, masks
    from concourse import bass_isa

    f32 = mybir.dt.float32
    Alu = mybir.AluOpType
    AF = mybir.ActivationFunctionType
    AX = mybir.AxisListType
    mmdt = mybir.dt.float32r if USE_F32R else f32

    nc = bacc.Bacc()
    Qd = nc.declare_dram_parameter("Q", [B, N, D], f32, isOutput=False)
    Kd = nc.declare_dram_parameter("K", [B, N, D], f32, isOutput=False)
    Od = nc.declare_dram_parameter("out", [B, N, N], f32, isOutput=True)

    with tile.TileContext(nc) as tc:
        with (
            tc.tile_pool(name="const", bufs=1) as constp,
            tc.tile_pool(name="ld", bufs=2) as ldp,
            tc.tile_pool(name="uv", bufs=2) as uvp,
            tc.tile_pool(name="sq", bufs=SQ_BUFS) as sqp,
            tc.tile_pool(name="dmy", bufs=2) as dmyp,
            tc.tile_pool(name="small", bufs=2) as smallp,
            tc.tile_pool(name="psd", bufs=3, space=bass.MemorySpace.PSUM) as psd,
            tc.tile_pool(name="pst", bufs=2, space=bass.MemorySpace.PSUM) as pst,
        ):
            ident = constp.tile([P, P], f32)
            masks.make_identity(nc, ident[:])

            for b in range(B):
                # ---------------- phase A: build UT / VT ----------------
                uts = []
                for (src, is_k) in ((Qd, False), (Kd, True)):
                    # cols 0:D = data, then for Q: col D = 1, col D+1 = q_sq
                    #                  for K: col D = k_sq, col D+1 = 1
                    # d2 = UT.T @ VT with VT = -2*[K^T; k_sq; 1] (copy scale -2)
                    # and UT = [Q^T; -1/2; -q_sq/2] (memset/TTR-scale -1/2).
                    ld = ldp.tile([P, NT, D + 2], f32, tag="ld")
                    nc.gpsimd.memset(ld[:], 1.0 if is_k else -0.5)
                    nc.sync.dma_start(
                        ld[:, :, 0:D], src[b].rearrange("(t p) d -> p t d", p=P)
                    )
                    sumcol = D if is_k else D + 1
                    TT = uvp.tile([D + 2, N], mmdt, tag="vt" if is_k else "ut")
                    for g in range(4):
                        ps = pst.tile([D + 2, 4 * P], f32, tag="tp")
                        for u in range(4):
                            t = g * 4 + u
                            if PHASE_MODE != "nottr":
                                # ld[:,t,sumcol] = scale * sum_d ld[:,t,d]^2
                                # (gpsimd square + DVE fused scale-sum; TTR
                                # with a broadcast out fails at runtime on HW)
                                sqld = dmyp.tile([P, D], f32, tag="sqld")
                                nc.gpsimd.tensor_tensor(
                                    sqld[:], ld[:, t, 0:D], ld[:, t, 0:D], Alu.mult
                                )
                                dmy = dmyp.tile([P, 1], f32, tag="dmy")
                                nc.vector.tensor_scalar(
                                    dmy[:].broadcast_to((P, D)),
                                    sqld[:],
                                    1.0 if is_k else -0.5,
                                    None,
                                    Alu.mult,
                                    Alu.add,
                                    accum_out=ld[:, t, sumcol : sumcol + 1],
                                )
                            nc.tensor.transpose(
                                ps[:, u * P : (u + 1) * P], ld[:, t, :], ident[:]
                            )
                        cols = slice(g * 4 * P, (g + 1) * 4 * P)
                        if is_k:
                            nc.scalar.mul(TT[:, cols], ps[:], -2.0)
                        else:
                            nc.scalar.copy(TT[:, cols], ps[:])
                    uts.append(TT)
                UT, VT = uts

                # ---------------- phase B: d2 -> sqrt -> min/max ----------------
                minp = smallp.tile([P, NT], f32, tag="minp")
                maxp = smallp.tile([P, NT], f32, tag="maxp")
                sqs = []
                for i in range(NT):
                    sq = sqp.tile([P, N], f32, tag="sq")
                    lhs = UT[:, i * P : (i + 1) * P]
                    for h in range(2):
                        d2 = psd.tile([P, HALF], f32, tag="d2")
                        for j in range(2):
                            c = h * 2 + j
                            nc.tensor.matmul(
                                d2[:, j * 512 : (j + 1) * 512],
                                lhs,
                                VT[:, c * 512 : (c + 1) * 512],
                                start=True,
                                stop=True,
                            )
                        nc.scalar.activation(
                            sq[:, h * HALF : (h + 1) * HALF], d2[:], AF.Sqrt
                        )
                    if PHASE_MODE in ("nored", "nottr"):
                        pass
                    elif FUSED_REDUCE:
                        # minp holds NEGATED per-tile minima (max of -sq), so
                        # both final reductions are max-reduces.
                        dm0 = dmyp.tile([P, 1], f32, tag="dmy")
                        nc.vector.tensor_scalar(
                            dm0[:].broadcast_to((P, N)),
                            sq[:],
                            -1.0,
                            None,
                            Alu.mult,
                            Alu.max,
                            accum_out=minp[:, i : i + 1],
                        )
                        dm1 = dmyp.tile([P, 1], f32, tag="dmy")
                        nc.vector.tensor_scalar(
                            dm1[:].broadcast_to((P, N)),
                            sq[:],
                            1.0,
                            None,
                            Alu.mult,
                            Alu.max,
                            accum_out=maxp[:, i : i + 1],
                        )
                    else:
                        nc.vector.tensor_reduce(
                            minp[:, i : i + 1], sq[:], AX.X, Alu.min
                        )
                        nc.vector.tensor_reduce(
                            maxp[:, i : i + 1], sq[:], AX.X, Alu.max
                        )
                    sqs.append(sq)

                # ---------------- phase C: finalize scalars ----------------
                if PHASE_MODE in ("nored", "nottr"):
                    for i in range(NT):
                        nc.sync.dma_start(Od[b, i * P : (i + 1) * P, :], sqs[i][:])
                    continue
                # s[:,0] = -min (via negated partials), s[:,1] = max; one
                # gpsimd all-reduce handles both (both are max-reduces).
                s2 = smallp.tile([P, 2], f32, tag="s2")
                sr = smallp.tile([P, 2], f32, tag="sr")
                u = smallp.tile([P, 1], f32, tag="u")
                r = smallp.tile([P, 1], f32, tag="r")
                c1 = smallp.tile([P, 1], f32, tag="c1")
                t0 = smallp.tile([P, 1], f32, tag="t0")
                c0 = smallp.tile([P, 1], f32, tag="c0")

                if FUSED_REDUCE:
                    nc.vector.tensor_reduce(s2[:, 0:1], minp[:], AX.X, Alu.max)
                else:
                    m1 = smallp.tile([P, 1], f32, tag="m1")
                    nc.vector.tensor_reduce(m1[:], minp[:], AX.X, Alu.min)
                    nc.vector.tensor_scalar(
                        s2[:, 0:1], m1[:], -1.0, None, Alu.mult
                    )
                nc.vector.tensor_reduce(s2[:, 1:2], maxp[:], AX.X, Alu.max)
                nc.gpsimd.partition_all_reduce(
                    sr[:], s2[:], P, bass_isa.ReduceOp.max
                )
                nmn = sr[:, 0:1]  # -min, on every partition
                mx = sr[:, 1:2]  # max, on every partition
                # c1 = (A_HI-A_LO)/(mx-mn);  c0 = A_LO - mn*c1 = A_LO + nmn*c1
                nc.vector.tensor_tensor(u[:], mx, nmn, Alu.add)  # mx - mn
                nc.vector.reciprocal(r[:], u[:])
                nc.vector.tensor_scalar(c1[:], r[:], A_HI - A_LO, None, Alu.mult)
                nc.vector.tensor_tensor(t0[:], nmn, c1[:], Alu.mult)
                nc.vector.tensor_scalar(c0[:], t0[:], A_LO, None, Alu.add)

                # ---------------- phase D: affine + store ----------------
                for i in range(NT):
                    sq = sqs[i]
                    if PHASE_MODE == "nomadd":
                        pass
                    elif i % NT < ACT_MADDS:
                        nc.scalar.activation(
                            sq[:],
                            sq[:],
                            AF.Identity,
                            bias=c0[:, 0:1],
                            scale=c1[:, 0:1],
                        )
                    else:
                        nc.vector.tensor_scalar(
                            sq[:], sq[:], c1[:, 0:1], c0[:, 0:1], Alu.mult, Alu.add
                        )
                    nc.sync.dma_start(Od[b, i * P : (i + 1) * P, :], sq[:])

    nc.compile()
    return nc


def get_program():
    if "nc" not in _CACHE:
        _CACHE["nc"] = build_program()
    return _CACHE["nc"]


def kernel(**inputs) -> np.ndarray:
    Q = np.ascontiguousarray(np.asarray(inputs["Q"], dtype=np.float32))
    K = np.ascontiguousarray(np.asarray(inputs["K"], dtype=np.float32))
    assert Q.shape == (H, B, N, D) and K.shape == (H, B, N, D)

    from concourse.bass_utils import run_bass_kernel_spmd

    nc = get_program()
    in_maps = [{"Q": Q[h], "K": K[h]} for h in range(H)]
    res = run_bass_kernel_spmd(nc, in_maps, core_ids=list(range(H)))
    out = np.stack([np.asarray(res.results[h]["out"]) for h in range(H)], axis=0)
    return out


if __name__ == "__main__":
    # quick smoke: build only
    nc = get_program()
    print("build ok:", nc)
